# revision 48
# baseline (speedup 1.0000x reference)
"""Multi-head attention (B=2, S=2048, D=1024, H=16) on 8 trn2 NeuronCores.

Sharding: core c -> batch b = c // 4, head group g = c % 4 (heads 4g..4g+3).
Each core computes, for its batch shard and 4 heads:
  QT/KT = (x W + b)^T in [d_local, seq] layout, V in [seq, d_local] layout,
  transposed scores S^T[k, q] = K Q^T (so softmax needs no transposes),
  exp via ACT (scale fused), PV matmul with an appended ones column which
  yields both the unnormalized context and the softmax row sums,
  normalization via a gpsimd partition-broadcast reciprocal multiply,
  and a partial output projection against a row shard of Wo.
Host sums the 4 partials per batch and adds the constant row bv @ Wo + bo
(softmax rows sum to one, so bv's contribution is a constant vector).

v2 schedule (engine-balance driven):
  The scalar engine's exp stream (128 ACTs x ~1.09us = 139us) and the PE
  column stream (166us bf16 / 138us with fp8-DoubleRow PV) are the two
  near-tied bottlenecks, so the schedule keeps both saturated:
  - x is DMAed in token halves; Q(query-half 0) and K(keys 0:1024) are
    projected per-chunk as DMAs land, so scores/exp start at ~7us instead
    of ~18us.
  - the attention loop runs over (query-half, head) phases of 16 key
    chunks each; PV pairs trail the exp stream, and each phase's last two
    PV pairs + normalize spill into the next phase so the PE never waits
    at phase boundaries.
  - remaining projections (K keys 1024:, dblk1 Q/K, query-half-1 Q) and
    the query-half-0 output projection are emitted as fillers inside
    ACT-bound phases.
  - PV runs in fp8e4m3 DoubleRow (2x PE throughput). exp is scaled by 16
    (bias ln16 folded into ACT) and V by 64 (folded into Wv host-side,
    ones column = 64) to lift fp8 operands out of the subnormal range;
    both factors cancel exactly in the softmax normalization.
"""

import sys

sys.path.insert(0, "/opt/trn_rl_repo")

import numpy as np
import ml_dtypes

B = 2
S = 2048
D = 1024
H = 16
HD = 64
NCORES = 8
HPC = 4          # heads per core
DL = HPC * HD    # 256 local head dims per core
P = 128
KCH = S // P     # 16 key chunks
DCH = D // P     # 8 contraction chunks
QB = S // 2      # query half size (1024)
SCALE = 1.0 / np.sqrt(HD)

# fp8 DoubleRow PV measured SLOWER than bf16 on this hardware (634-987ns
# vs 379ns per 512-col matmul; the sparser PE stream also trips the HAM
# half-speed throttle) and costs ~1.1e-2 extra error. Keep bf16.
USE_FP8_PV = True
EXP_SCALE = 16.0 if USE_FP8_PV else 1.0   # folded into ACT bias
V_SCALE = 64.0 if USE_FP8_PV else 1.0     # folded into Wv host-side

_CACHE = {}


def _build():
    import concourse.bass as bass  # noqa: F401
    import concourse.mybir as mybir
    import concourse.tile as tile
    from concourse import bacc

    bf16 = mybir.dt.bfloat16
    f32 = mybir.dt.float32
    fp8 = mybir.dt.float8e4
    DR = mybir.MatmulPerfMode.DoubleRow
    Exp = mybir.ActivationFunctionType.Exp
    att_dt = fp8 if USE_FP8_PV else bf16
    exp_bias = float(np.log(EXP_SCALE))

    nc = bacc.Bacc("TRN2", target_bir_lowering=False, debug=False,
                   num_devices=NCORES)

    # weights are host-packed partition-major ([128, chunks*cols]) so the
    # DMAs move 4KB contiguous lines per partition at full HBM speed
    xT_d = nc.dram_tensor("xt", [D, S], bf16, kind="ExternalInput")
    wq_d = nc.dram_tensor("wq", [P, DCH * DL], bf16, kind="ExternalInput")
    wk_d = nc.dram_tensor("wk", [P, DCH * DL], bf16, kind="ExternalInput")
    wv_d = nc.dram_tensor("wv", [P, DCH * DL], bf16, kind="ExternalInput")
    wo_d = nc.dram_tensor("wo", [P, 2 * D], bf16, kind="ExternalInput")
    bqk_d = nc.dram_tensor("bqk", [P, 4], f32, kind="ExternalInput")
    out_d = nc.dram_tensor("out", [S, D], bf16, kind="ExternalOutput")

    with tile.TileContext(nc) as tc:
        with (
            tc.tile_pool(name="persist", bufs=1) as pp,
            tc.tile_pool(name="stream", bufs=3) as sp,
            tc.tile_pool(name="psum", bufs=1, space="PSUM") as ps,
        ):
            # ---- persistent SBUF tiles
            bqk_s = pp.tile([P, 4], f32, tag="bqk", name="bqk_s")
            wq_s = pp.tile([P, 2, DCH, P], bf16, tag="wq", name="wq_s")
            wk_s = pp.tile([P, 2, DCH, P], bf16, tag="wk", name="wk_s")
            wv_s = pp.tile([P, DCH, DL], bf16, tag="wv", name="wv_s")
            wo_s = pp.tile([P, 2, D], bf16, tag="wo", name="wo_s")
            xts = [pp.tile([P, S], bf16, tag=f"xt{c}", name=f"xt{c}")
                   for c in range(DCH)]
            qt = [pp.tile([P, S], bf16, tag=f"qt{d}", name=f"qt{d}")
                  for d in range(2)]
            kt = [pp.tile([P, S], bf16, tag=f"kt{d}", name=f"kt{d}")
                  for d in range(2)]
            ctx_sb = [pp.tile([P, S], bf16, tag=f"ctx{d}", name=f"ctx{d}")
                      for d in range(2)]
            # V in k-chunk pairs: [128, 2, 4 heads * 68]; col 68h+64 holds
            # the softmax-sum column (= V_SCALE to cancel the Wv scaling).
            vts = [pp.tile([P, 2, HPC * 68], att_dt, tag=f"v{pr}",
                           name=f"v{pr}") for pr in range(KCH // 2)]

            # ---- DMA emission (per-queue order == landing order)
            # q0 = sync, q1 = gpsimd. x lands in token halves so the front
            # projections can start after ~3MB instead of ~5MB.
            nc.sync.dma_start(bqk_s[:], bqk_d[:])
            # q0: wq dblk0, even x half0 chunks, wv, x half1, wq dblk1;
            # q1: wk dblk0, odd chunks, ..., wk dblk1, wo. Only 256KB of
            # weights precede x on each queue; token half 0 (2MB) lands
            # first so scores/exp start ~12us earlier, half 1 feeds the
            # K-half1 projection and V token blocks 8-15 mid-phase-0.
            nc.sync.dma_start(
                wq_s[:, 0].rearrange("p c d -> p (c d)"), wq_d[:, 0:1024])
            nc.gpsimd.dma_start(
                wk_s[:, 0].rearrange("p c d -> p (c d)"), wk_d[:, 0:1024])
            for c in range(DCH):
                eng = nc.sync if c % 2 == 0 else nc.gpsimd
                eng.dma_start(xts[c][:, 0:QB], xT_d[c * P:(c + 1) * P, 0:QB])
            nc.sync.dma_start(
                wv_s[:, 0:4, :].rearrange("p c d -> p (c d)"),
                wv_d[:, 0:4 * DL])
            nc.gpsimd.dma_start(
                wv_s[:, 4:8, :].rearrange("p c d -> p (c d)"),
                wv_d[:, 4 * DL:])
            for c in range(DCH):
                eng = nc.sync if c % 2 == 0 else nc.gpsimd
                eng.dma_start(xts[c][:, QB:S], xT_d[c * P:(c + 1) * P, QB:S])
            nc.sync.dma_start(
                wq_s[:, 1].rearrange("p c d -> p (c d)"), wq_d[:, 1024:2048])
            nc.gpsimd.dma_start(
                wk_s[:, 1].rearrange("p c d -> p (c d)"), wk_d[:, 1024:2048])
            nc.gpsimd.dma_start(
                wo_s[:].rearrange("p c d -> p (c d)"), wo_d[:])

            # ones columns for the PV sum trick
            for pr in range(KCH // 2):
                v4 = vts[pr].rearrange("p j (h e) -> p j h e", e=68)
                nc.vector.memset(v4[:, :, :, 64:65], float(V_SCALE))
            # exp bias tile (ln EXP_SCALE folded into the ACT)
            ebias = pp.tile([P, 1], f32, tag="ebias", name="ebias")
            nc.vector.memset(ebias[:], exp_bias)

            # ---- PE warm-up: dummy matmuls on a zeroed tile keep the PE
            # busy through the x-DMA wait so the HAM p-state is at full
            # clock when the real front projections land.
            junk = sp.tile([64, 512], bf16, tag="junk", bufs=1, name="junk")
            nc.vector.memset(junk[:], 0.0)
            warm_ps = ps.tile([P, 1024], f32, tag="sc", bufs=2,
                              name="ps_warm")

            def junk_mm(n):
                for i in range(n):
                    nc.tensor.matmul(warm_ps[:, 0:512], junk[:, 0:128],
                                     junk[:, 0:512], start=True, stop=True)
            junk_mm(6)

            # ---- helpers -------------------------------------------------
            def make_proj(which, dblk, c0, c1, acc_tag, add_eng="vector"):
                """Project tokens [c0:c1) of Q or K for dblk. Returns
                (emit(kcs), finish()); the psum acc is allocated lazily at
                the first emit so pool rotation follows emission order."""
                w_s, bcol = (wq_s, 0) if which == 0 else (wk_s, 2)
                t_sb = (qt if which == 0 else kt)[dblk]
                width = c1 - c0
                st = {}

                def emit(kcs):
                    if "acc" not in st:
                        st["acc"] = ps.tile(
                            [P, 1024], f32, tag=acc_tag,
                            bufs=2 if acc_tag == "sc" else 1,
                            name=f"ps_p{which}{dblk}_{c0}")
                    acc = st["acc"]
                    for kc in kcs:
                        for ns in range(width // 512):
                            nc.tensor.matmul(
                                acc[:, ns * 512:(ns + 1) * 512],
                                w_s[:, dblk, kc, :],
                                xts[kc][:, c0 + ns * 512:
                                        c0 + (ns + 1) * 512],
                                start=(kc == 0), stop=(kc == DCH - 1),
                            )

                def finish():
                    if add_eng == "scalar":
                        nc.scalar.add(t_sb[:, c0:c1], st["acc"][:, 0:width],
                                      bqk_s[:, bcol + dblk:bcol + dblk + 1])
                    else:
                        nc.vector.tensor_scalar_add(
                            t_sb[:, c0:c1], st["acc"][:, 0:width],
                            bqk_s[:, bcol + dblk:bcol + dblk + 1],
                        )
                return emit, finish

            def filler_proj(which, dblk, c0, c1, parts=1):
                """Filler projection in `parts` bursts + bias add. The
                bursts must be contiguous uses of the aux psum buffer (an
                interleaved aux user would clobber the accumulation)."""
                emit, fin = make_proj(which, dblk, c0, c1, "aux")
                step = DCH // parts

                def mk(p):
                    def burst():
                        emit(range(p * step, (p + 1) * step))
                        if p == parts - 1:
                            fin()
                    return burst
                return [mk(p) for p in range(parts)]

            def v_proj(tb):
                pr, j = tb // 2, tb % 2
                v4 = vts[pr].rearrange("p j (h e) -> p j h e", e=68)
                acc = ps.tile([P, 1024], f32, tag="aux", name=f"ps_v{tb}")
                for kc in range(DCH):
                    nc.tensor.matmul(
                        acc[:, 0:DL],
                        xts[kc][:, tb * P:(tb + 1) * P],
                        wv_s[:, kc, :],
                        start=(kc == 0), stop=(kc == DCH - 1),
                    )
                nc.vector.tensor_copy(
                    v4[:, j, :, 0:64],
                    acc[:, 0:DL].rearrange("p (h e) -> p h e", e=64),
                )

            # ---- attention phase machinery ------------------------------
            NPAIR = KCH // 2
            etps = [None] * NPAIR

            def scores_chunk(state, kc):
                h, qh = state["h"], state["qh"]
                dblk = h // 2
                roff = 64 * (h % 2)
                pr, j = kc // 2, kc % 2
                if j == 0:
                    etps[pr] = sp.tile([P, 2, QB], att_dt, tag="expt",
                                       bufs=6, name=f"et{qh}{h}_{pr}")
                et = etps[pr]
                sc = ps.tile([P, 1024], f32, tag="sc", bufs=2,
                             name=f"ps_sc{qh}{h}_{kc}")
                for ns in range(2):
                    nc.tensor.matmul(
                        sc[:, ns * 512:(ns + 1) * 512],
                        kt[dblk][roff:roff + 64, kc * P:(kc + 1) * P],
                        qt[dblk][roff:roff + 64,
                                 qh * QB + ns * 512:qh * QB + (ns + 1) * 512],
                        start=True, stop=True,
                    )
                nc.scalar.activation(et[:, j, :], sc[:], Exp,
                                     bias=ebias[:], scale=float(SCALE))
                state.setdefault("ets", {})[pr] = et

            def pv_pair(state, pr):
                h = state["h"]
                if pr == 0:
                    state["ctx_ps"] = ps.tile(
                        [P, 1024], f32, tag="ctx", bufs=1,
                        name=f"ps_ctx{state['qh']}{h}")
                ctx_ps = state["ctx_ps"]
                et = state["ets"][pr]
                v4 = vts[pr].rearrange("p j (h e) -> p j h e", e=68)
                if USE_FP8_PV:
                    for ns in range(2):
                        nc.tensor.matmul(
                            ctx_ps[0:65, ns * 512:(ns + 1) * 512],
                            v4[:, :, h, 0:65],
                            et[:, :, ns * 512:(ns + 1) * 512],
                            start=(pr == 0), stop=(pr == NPAIR - 1),
                            perf_mode=DR,
                        )
                else:
                    for j in range(2):
                        for ns in range(2):
                            nc.tensor.matmul(
                                ctx_ps[0:65, ns * 512:(ns + 1) * 512],
                                v4[:, j, h, 0:65],
                                et[:, j, ns * 512:(ns + 1) * 512],
                                start=(pr == 0 and j == 0),
                                stop=(pr == NPAIR - 1 and j == 1),
                            )

            def norm_pre(state, part, nparts):
                """Sum-row copy + reciprocal + broadcast for one slice."""
                h, qh = state["h"], state["qh"]
                w = QB // nparts
                ctx_ps = state["ctx_ps"]
                hs = slice(part * w, (part + 1) * w)
                srow = sp.tile([1, w], f32, tag=f"srow{w}", bufs=4,
                               name=f"srow{qh}{h}_{part}")
                nc.vector.tensor_copy(srow[:], ctx_ps[64:65, hs])
                rec = sp.tile([1, w], f32, tag=f"rec{w}", bufs=4,
                              name=f"rec{qh}{h}_{part}")
                nc.vector.reciprocal_approx_fast(rec[:], srow[:])
                bc = sp.tile([64, w], f32, tag=f"bc{w}", bufs=4,
                             name=f"bc{qh}{h}_{part}")
                nc.gpsimd.partition_broadcast(bc[:], rec[:])
                return hs, bc

            def norm_mult(state, hs, bc):
                h, qh = state["h"], state["qh"]
                dblk = h // 2
                roff = 64 * (h % 2)
                nc.vector.tensor_mul(
                    ctx_sb[dblk][roff:roff + 64, qh * QB + hs.start:
                                 qh * QB + hs.stop],
                    state["ctx_ps"][0:64, hs], bc[:])

            def normalize(state, part, nparts):
                hs, bc = norm_pre(state, part, nparts)
                norm_mult(state, hs, bc)

            def out_tb(tb, acc_tag, copy_eng):
                """Output projection for global token block tb."""
                acc = ps.tile([P, 1024], f32, tag=acc_tag,
                              bufs=2 if acc_tag == "sc" else 1,
                              name=f"ps_o{tb}")
                for dc in range(2):
                    for ns in range(2):
                        nc.tensor.matmul(
                            acc[:, ns * 512:(ns + 1) * 512],
                            ctx_sb[dc][:, tb * P:(tb + 1) * P],
                            wo_s[:, dc, ns * 512:(ns + 1) * 512],
                            start=(dc == 0), stop=(dc == 1),
                        )
                o_sb = sp.tile([P, D], bf16, tag="osb", name=f"osb{tb}")
                if copy_eng == "scalar":
                    nc.scalar.copy(o_sb[:], acc[:])
                else:
                    nc.vector.tensor_copy(o_sb[:], acc[:])
                nc.sync.dma_start(out_d[tb * P:(tb + 1) * P, :], o_sb[:])

            # ---- front: Q(qh0) + K(keys 0:1024) per-chunk as x lands;
            # junk matmuls between chunks keep the PE streak (and thus
            # the p-state ramp) alive while DMA-bound.
            junk_mm(10)
            q0_emit, q0_fin = make_proj(0, 0, 0, QB, "aux",
                                        add_eng="scalar")
            ka_emit, ka_fin = make_proj(1, 0, 0, QB, "ctx")
            for kc in range(DCH):
                q0_emit([kc])
                ka_emit([kc])
                junk_mm(1)
            q0_fin()
            ka_fin()

            # ---- filler schedule: phase index (qh*4+h) -> {kc: [thunks]}
            fill = {ph: {} for ph in range(8)}

            def add_fill(ph, kc, thunk):
                fill[ph].setdefault(kc, []).append(thunk)

            # phase 0 (qh0,h0): V projection (tb 8-15 and the K keys
            # 1024:2048 bursts wait on x token-half 1; the first K burst
            # sits at kc4 so its keys are ready before this phase's kc8)
            for tb in range(16):
                add_fill(0, tb, lambda tb=tb: v_proj(tb))
            add_fill(0, 4, filler_proj(1, 0, QB, QB + 512)[0])
            add_fill(0, 6, filler_proj(1, 0, QB + 512, S)[0])
            # phase 1 (qh0,h1): K dblk1 keys 0:1024, Q dblk1 qh0
            for i, th in enumerate(filler_proj(1, 1, 0, QB, parts=2)):
                add_fill(1, 5 + 2 * i, th)
            for i, th in enumerate(filler_proj(0, 1, 0, QB, parts=2)):
                add_fill(1, 10 + 2 * i, th)
            # phase 2 (qh0,h2): K dblk1 keys 1024:2048 (before own kc8)
            for i, th in enumerate(filler_proj(1, 1, QB, S, parts=2)):
                add_fill(2, 4 + 2 * i, th)
            # phase 3 (qh0,h3): Q dblk0 qh1
            for i, th in enumerate(filler_proj(0, 0, QB, S, parts=2)):
                add_fill(3, 5 + 2 * i, th)
            # phase 4 (qh1,h0): Q dblk1 qh1
            for i, th in enumerate(filler_proj(0, 1, QB, S, parts=2)):
                add_fill(4, 5 + 2 * i, th)
            # phases 5-7 (qh1,h1-h3): out-proj qh0 tb0-7 spread to keep
            # the otherwise ACT-bound late phases PE-dense
            out_spots = [(5, 5, 0), (5, 10, 1), (6, 4, 2), (6, 8, 3),
                         (6, 12, 4), (7, 4, 5), (7, 8, 6), (7, 12, 7)]
            for ph, kc, tb in out_spots:
                add_fill(ph, kc, lambda tb=tb: out_tb(tb, "aux", "vector"))

            # PV stagger: pairs 0-3 late in their own phase; pairs 4-7 +
            # normalize spill into the next phase's kc0-3 so PV work sits
            # right behind the ACT wavefront and the PE never waits on
            # exp results. The last phase keeps more pairs in-phase.
            pv_slot = {9: 0, 11: 1, 13: 2, 15: 3}
            pv_slot_last = {5: 0, 7: 1, 9: 2, 11: 3, 13: 4, 15: 5}

            prev = None
            for ph in range(8):
                qh, h = ph // 4, ph % 4
                state = {"h": h, "qh": qh}
                slots = pv_slot_last if ph == 7 else pv_slot
                for kc in range(KCH):
                    scores_chunk(state, kc)
                    # spilled tail of the previous phase
                    if prev is not None and kc < 4:
                        pv_pair(prev, NPAIR - 4 + kc)
                        if kc == 3:
                            normalize(prev, 0, 2)
                            normalize(prev, 1, 2)
                    for th in fill[ph].get(kc, []):
                        th()
                    if kc in slots:
                        pv_pair(state, slots[kc])
                prev = state

            # ---- tail: last phase PV, then all reciprocal/broadcast
            # chains (vector runs them back to back without blocking on
            # gpsimd), then per chunk: multiply + out-projection. Copies
            # alternate between the idle scalar engine and vector.
            pv_pair(prev, NPAIR - 2)
            pv_pair(prev, NPAIR - 1)
            pre = [norm_pre(prev, c, 8) for c in range(8)]
            for c in range(8):
                norm_mult(prev, *pre[c])
                tb = 8 + c
                out_tb(tb, "sc" if tb % 2 else "aux",
                       "scalar" if c % 2 == 0 else "vector")

    nc.compile()
    return nc


def _get_compiled():
    if "nc" not in _CACHE:
        _CACHE["nc"] = _build()
    return _CACHE["nc"]


def kernel(x, Wq, bq, Wk, bk, Wv, bv, Wo, bo):
    from concourse.bass_utils import run_bass_kernel_spmd

    nc = _get_compiled()
    x = np.asarray(x, dtype=np.float32)
    Wq, bq = np.asarray(Wq, np.float32), np.asarray(bq, np.float32)
    Wk, bk = np.asarray(Wk, np.float32), np.asarray(bk, np.float32)
    Wv, bv = np.asarray(Wv, np.float32), np.asarray(bv, np.float32)
    Wo, bo = np.asarray(Wo, np.float32), np.asarray(bo, np.float32)

    bf = ml_dtypes.bfloat16
    in_maps = []
    for c in range(NCORES):
        b, g = c // 4, c % 4
        cols = slice(g * DL, (g + 1) * DL)
        bq_l, bk_l = bq[cols], bk[cols]
        bqk = np.stack(
            [bq_l[0:P], bq_l[P:2 * P], bk_l[0:P], bk_l[P:2 * P]], axis=1)
        def pack(w):  # [C*128, d] -> [128, C*d] partition-major
            c, d = w.shape[0] // P, w.shape[1]
            return np.ascontiguousarray(
                w.reshape(c, P, d).transpose(1, 0, 2).reshape(P, c * d))

        def pack_qk(w):  # [1024, 256] -> [128, dblk*1024 + c*128 + d]
            return np.ascontiguousarray(
                w.reshape(DCH, P, 2, P).transpose(1, 2, 0, 3).reshape(
                    P, 2 * DCH * P))

        in_maps.append({
            "xt": np.ascontiguousarray(x[b].T).astype(bf),
            "wq": pack_qk(Wq[:, cols]).astype(bf),
            "wk": pack_qk(Wk[:, cols]).astype(bf),
            "wv": pack(Wv[:, cols] * V_SCALE).astype(bf),
            "wo": pack(Wo[cols, :]).astype(bf),
            "bqk": np.ascontiguousarray(bqk, np.float32),
        })

    _CACHE["in_maps"] = in_maps
    res = run_bass_kernel_spmd(nc, in_maps, list(range(NCORES)))

    # constant row: bv @ Wo + bo (softmax rows sum to 1)
    const_row = bv.astype(np.float64) @ Wo.astype(np.float64) + bo
    out = np.zeros((B, S, D), np.float64)
    for c in range(NCORES):
        out[c // 4] += res.results[c]["out"].astype(np.float64)
    out += const_row
    return out.astype(np.float32)
